# revision 1
# baseline (speedup 1.0000x reference)
"""DeepIRT (DKVMN) Trainium2 kernel — data-parallel over batch on 8 NeuronCores.

Self-contained: builds the Bass program on first call, shards the batch
2048 -> 8 x 256, runs via run_bass_kernel_spmd, reassembles full outputs.
Returns (p, theta, beta, alpha, z), each [2048, 200] float32, matching the
reference's tuple output.
"""
import sys

for _p in ("/opt/trn_rl_repo", "/root/.axon_site/_ro/trn_rl_repo"):
    if _p not in sys.path:
        sys.path.append(_p)

import numpy as np
from contextlib import ExitStack

import concourse.bacc as bacc
import concourse.bass as bass
import concourse.tile as tile
from concourse import mybir
from concourse.masks import make_identity
from concourse.bass_utils import run_bass_kernel_spmd

f32 = mybir.dt.float32
i32 = mybir.dt.int32
AF = mybir.ActivationFunctionType
OP = mybir.AluOpType

B, S = 2048, 200
NQ = 10000
M, K, V, F = 50, 50, 200, 50
ABILITY_SCALE = 3.0
NCORES = 8
BC = B // NCORES
NTL = BC // 128


def _build_program(S=S, num_devices=NCORES):
    nc = bacc.Bacc("TRN2", target_bir_lowering=False, debug=False,
                   num_devices=num_devices)

    q_idx = nc.dram_tensor("q_idx", [BC, S], i32, kind="ExternalInput")
    qa_idx = nc.dram_tensor("qa_idx", [BC, S], i32, kind="ExternalInput")
    q_emb = nc.dram_tensor("q_emb", [NQ + 1, K], f32, kind="ExternalInput")
    qa_emb = nc.dram_tensor("qa_emb", [2 * NQ + 1, V], f32, kind="ExternalInput")
    init_vm = nc.dram_tensor("init_vm", [1, V * M], f32, kind="ExternalInput")
    eaW = nc.dram_tensor("eaW", [V, 2 * V], f32, kind="ExternalInput")
    eab = nc.dram_tensor("eab", [1, 2 * V], f32, kind="ExternalInput")
    kb = nc.dram_tensor("kb", [K, M + 1], f32, kind="ExternalInput")
    kbb = nc.dram_tensor("kbb", [1, M + 1], f32, kind="ExternalInput")
    sWr = nc.dram_tensor("sWr", [V, F], f32, kind="ExternalInput")
    sWq = nc.dram_tensor("sWq", [K, F], f32, kind="ExternalInput")
    sbias = nc.dram_tensor("sbias", [1, F], f32, kind="ExternalInput")
    thaW = nc.dram_tensor("thaW", [F, 2], f32, kind="ExternalInput")
    aWq2 = nc.dram_tensor("aWq2", [K, 2], f32, kind="ExternalInput")
    hb = nc.dram_tensor("hb", [1, 2], f32, kind="ExternalInput")

    p_o = nc.dram_tensor("p_o", [BC, S], f32, kind="ExternalOutput")
    th_o = nc.dram_tensor("th_o", [BC, S], f32, kind="ExternalOutput")
    be_o = nc.dram_tensor("be_o", [BC, S], f32, kind="ExternalOutput")
    al_o = nc.dram_tensor("al_o", [BC, S], f32, kind="ExternalOutput")
    z_o = nc.dram_tensor("z_o", [BC, S], f32, kind="ExternalOutput")

    with ExitStack() as ctx:
        tc = ctx.enter_context(tile.TileContext(nc))

        const_p = ctx.enter_context(tc.tile_pool(name="const", bufs=1))
        state_p = ctx.enter_context(tc.tile_pool(name="state", bufs=1))
        dram_p = ctx.enter_context(tc.tile_pool(name="dramp", bufs=1, space="DRAM"))

        ident = const_p.tile([128, 128], f32)
        make_identity(nc, ident[:])
        ones_c = const_p.tile([1, 512], f32)
        nc.gpsimd.memset(ones_c[:], 1.0)

        eaW_a = const_p.tile([128, 2 * V], f32)
        nc.sync.dma_start(eaW_a[:], eaW[0:128, :])
        eaW_b = const_p.tile([128, 2 * V], f32)
        nc.sync.dma_start(eaW_b[: V - 128, :], eaW[128:V, :])
        eab_sb = const_p.tile([1, 2 * V], f32)
        nc.sync.dma_start(eab_sb[:], eab[:])
        kb_sb = const_p.tile([K, M + 1], f32)
        nc.sync.dma_start(kb_sb[:], kb[:])
        kbb_sb = const_p.tile([1, M + 1], f32)
        nc.sync.dma_start(kbb_sb[:], kbb[:])
        sWr_a = const_p.tile([128, F], f32)
        nc.sync.dma_start(sWr_a[:], sWr[0:128, :])
        sWr_b = const_p.tile([128, F], f32)
        nc.sync.dma_start(sWr_b[: V - 128, :], sWr[128:V, :])
        sWq_sb = const_p.tile([K, F], f32)
        nc.sync.dma_start(sWq_sb[:], sWq[:])
        sb_sb = const_p.tile([1, F], f32)
        nc.sync.dma_start(sb_sb[:], sbias[:])
        thaW_sb = const_p.tile([F, 2], f32)
        nc.sync.dma_start(thaW_sb[:], thaW[:])
        aWq2_sb = const_p.tile([K, 2], f32)
        nc.sync.dma_start(aWq2_sb[:], aWq2[:])
        hb_sb = const_p.tile([1, 2], f32)
        nc.sync.dma_start(hb_sb[:], hb[:])

        idxq = const_p.tile([128, NTL, S], i32)
        nc.sync.dma_start(idxq[:], q_idx[:].rearrange("(tl b) s -> b tl s", tl=NTL))
        idxqa = const_p.tile([128, NTL, S], i32)
        nc.sync.dma_start(idxqa[:], qa_idx[:].rearrange("(tl b) s -> b tl s", tl=NTL))

        mv = state_p.tile([128, NTL, V, M], f32)
        nc.sync.dma_start(
            mv[:],
            init_vm[:].rearrange("o (v m) -> o v m", m=M).unsqueeze(1)
            .broadcast_to([128, NTL, V, M]),
        )
        beta_sb = state_p.tile([128, NTL, S], f32)
        thal_sb = state_p.tile([128, NTL, S, 2], f32)
        scr = state_p.tile([128, V, M], f32)

        readT_d = dram_p.tile([S, V, BC], f32)
        qeT_d = dram_p.tile([S, K, BC], f32)

        ps_tp = ctx.enter_context(tc.tile_pool(name="ps_tp", bufs=2, space="PSUM"))
        rd_p = ctx.enter_context(tc.tile_pool(name="rd", bufs=4))
        ph12 = ExitStack()
        ph1_p = ph12.enter_context(tc.tile_pool(name="ph1", bufs=3))
        gates_p = ph12.enter_context(tc.tile_pool(name="gates", bufs=6))
        ps_ea = ph12.enter_context(tc.tile_pool(name="ps_ea", bufs=2, space="PSUM"))
        ps_lg = ph12.enter_context(tc.tile_pool(name="ps_lg", bufs=2, space="PSUM"))

        for t in range(S):
            gates = []
            for tl in range(NTL):
                qe_g = ph1_p.tile([128, K], f32, tag="qe_g")
                nc.gpsimd.indirect_dma_start(
                    out=qe_g[:], out_offset=None, in_=q_emb[:],
                    in_offset=bass.IndirectOffsetOnAxis(ap=idxq[:, tl, t:t + 1], axis=0),
                )
                qae_g = ph1_p.tile([128, V], f32, tag="qae_g")
                nc.gpsimd.indirect_dma_start(
                    out=qae_g[:], out_offset=None, in_=qa_emb[:],
                    in_offset=bass.IndirectOffsetOnAxis(ap=idxqa[:, tl, t:t + 1], axis=0),
                )
                tp1 = ps_tp.tile([128, 128], f32, tag="tp", space="PSUM")
                nc.tensor.transpose(out=tp1[:], in_=qae_g[:, 0:128], identity=ident[:])
                qaeT_a = ph1_p.tile([128, 128], f32, tag="qaeT_a")
                nc.scalar.copy(qaeT_a[:], tp1[:])
                tp2 = ps_tp.tile([128, 128], f32, tag="tp", space="PSUM")
                nc.tensor.transpose(out=tp2[: V - 128, :], in_=qae_g[:, 128:V],
                                    identity=ident[:])
                qaeT_b = ph1_p.tile([128, 128], f32, tag="qaeT_b")
                nc.scalar.copy(qaeT_b[: V - 128, :], tp2[: V - 128, :])
                tp3 = ps_tp.tile([128, 128], f32, tag="tp", space="PSUM")
                nc.tensor.transpose(out=tp3[:K, :], in_=qe_g[:], identity=ident[:])
                qeT_sb = ph1_p.tile([K, 128], f32, tag="qeT_sb")
                nc.scalar.copy(qeT_sb[:], tp3[:K, :])
                nc.sync.dma_start(qeT_d[t, :, tl * 128:(tl + 1) * 128], qeT_sb[:])

                ea_ps = ps_ea.tile([128, 2 * V], f32, tag="ea", space="PSUM")
                nc.tensor.matmul(out=ea_ps[:], lhsT=qaeT_a[:], rhs=eaW_a[:],
                                 start=True, stop=False)
                nc.tensor.matmul(out=ea_ps[:], lhsT=qaeT_b[: V - 128, :],
                                 rhs=eaW_b[: V - 128, :], start=False, stop=False)
                nc.tensor.matmul(out=ea_ps[:], lhsT=ones_c[:, 0:128],
                                 rhs=eab_sb[:], start=False, stop=True)
                e_t = gates_p.tile([128, V], f32, tag="e_t")
                nc.scalar.activation(e_t[:], ea_ps[:, 0:V], AF.Sigmoid)
                a_t = gates_p.tile([128, V], f32, tag="a_t")
                nc.scalar.activation(a_t[:], ea_ps[:, V:2 * V], AF.Tanh)

                # softmax via exp(x) = (1+tanh(x/2))/(1-tanh(x/2)); logits are
                # O(0.3) so no max-subtraction needed. Keeps ACT on the
                # sigmoid/tanh table set (no per-step table switches).
                lg_ps = ps_lg.tile([128, M + 1], f32, tag="lg", space="PSUM")
                nc.tensor.matmul(out=lg_ps[:], lhsT=qeT_sb[:], rhs=kb_sb[:],
                                 start=True, stop=False)
                nc.tensor.matmul(out=lg_ps[:], lhsT=ones_c[:, 0:128],
                                 rhs=kbb_sb[:], start=False, stop=True)
                th_t = rd_p.tile([128, M], f32, tag="th_t")
                nc.scalar.activation(th_t[:], lg_ps[:, 0:M], AF.Tanh, scale=0.5)
                num_t = rd_p.tile([128, M], f32, tag="num_t")
                nc.vector.tensor_scalar_add(num_t[:], th_t[:], 1.0)
                den_t = rd_p.tile([128, M], f32, tag="den_t")
                nc.vector.tensor_scalar(out=den_t[:], in0=th_t[:], scalar1=-1.0,
                                        scalar2=1.0, op0=OP.mult, op1=OP.add)
                w_t = gates_p.tile([128, M], f32, tag="w_t")
                nc.vector.reciprocal(den_t[:], den_t[:])
                nc.vector.tensor_tensor(out=w_t[:], in0=num_t[:], in1=den_t[:],
                                        op=OP.mult)
                sm = rd_p.tile([128, 1], f32, tag="sm")
                nc.vector.tensor_reduce(out=sm[:], in_=w_t[:],
                                        axis=mybir.AxisListType.X, op=OP.add)
                rs = rd_p.tile([128, 1], f32, tag="rs")
                nc.vector.reciprocal(rs[:], sm[:])
                nc.vector.tensor_scalar_mul(w_t[:], w_t[:], rs[:])
                nc.scalar.activation(beta_sb[:, tl, t:t + 1], lg_ps[:, M:M + 1],
                                     AF.Tanh)
                gates.append((e_t, a_t, w_t))

            for tl in range(NTL):
                e_t, a_t, w_t = gates[tl]
                mvv = mv[:, tl]
                e_bc = e_t[:].unsqueeze(2).broadcast_to([128, V, M])
                a_bc = a_t[:].unsqueeze(2).broadcast_to([128, V, M])
                w_bc = w_t[:].unsqueeze(1).broadcast_to([128, V, M])
                nc.vector.tensor_tensor(out=scr[:], in0=mvv, in1=e_bc, op=OP.mult)
                nc.vector.tensor_tensor(out=scr[:], in0=scr[:], in1=a_bc,
                                        op=OP.subtract)
                nc.vector.tensor_tensor(out=scr[:], in0=scr[:], in1=w_bc, op=OP.mult)
                R_t = rd_p.tile([128, V], f32, tag="R_t")
                nc.vector.tensor_reduce(out=R_t[:], in_=scr[:],
                                        axis=mybir.AxisListType.X, op=OP.add)
                nc.vector.tensor_tensor(out=mvv, in0=mvv, in1=scr[:], op=OP.subtract)
                rd_t = rd_p.tile([128, V], f32, tag="rd_t")
                nc.vector.tensor_tensor(out=rd_t[:], in0=R_t[:], in1=a_t[:], op=OP.add)
                re_t = rd_p.tile([128, V], f32, tag="re_t")
                nc.vector.reciprocal(re_t[:], e_t[:])
                nc.vector.tensor_tensor(out=rd_t[:], in0=rd_t[:], in1=re_t[:],
                                        op=OP.mult)
                tp4 = ps_tp.tile([128, 128], f32, tag="tp", space="PSUM")
                nc.tensor.transpose(out=tp4[:], in_=rd_t[:, 0:128], identity=ident[:])
                rT_a = rd_p.tile([128, 128], f32, tag="rT_a")
                nc.scalar.copy(rT_a[:], tp4[:])
                nc.sync.dma_start(readT_d[t, 0:128, tl * 128:(tl + 1) * 128], rT_a[:])
                tp5 = ps_tp.tile([128, 128], f32, tag="tp", space="PSUM")
                nc.tensor.transpose(out=tp5[: V - 128, :], in_=rd_t[:, 128:V],
                                    identity=ident[:])
                rT_b = rd_p.tile([128, 128], f32, tag="rT_b")
                nc.scalar.copy(rT_b[: V - 128, :], tp5[: V - 128, :])
                nc.sync.dma_start(readT_d[t, 128:V, tl * 128:(tl + 1) * 128],
                                  rT_b[: V - 128, :])

        ph12.close()
        p3_p = ctx.enter_context(tc.tile_pool(name="p3", bufs=3))
        ps_s = ctx.enter_context(tc.tile_pool(name="ps_s", bufs=2, space="PSUM"))
        for t in range(S):
            rTa_in = p3_p.tile([128, BC], f32, tag="rTa_in")
            nc.sync.dma_start(rTa_in[:], readT_d[t, 0:128, :])
            rTb_in = p3_p.tile([128, BC], f32, tag="rTb_in")
            nc.sync.dma_start(rTb_in[: V - 128, :], readT_d[t, 128:V, :])
            qeT_in = p3_p.tile([K, BC], f32, tag="qeT_in")
            nc.sync.dma_start(qeT_in[:], qeT_d[t])

            s_ps = ps_s.tile([F, BC], f32, tag="s_ps", space="PSUM")
            nc.tensor.matmul(out=s_ps[:], lhsT=sWr_a[:], rhs=rTa_in[:],
                             start=True, stop=False)
            nc.tensor.matmul(out=s_ps[:], lhsT=sWr_b[: V - 128, :],
                             rhs=rTb_in[: V - 128, :], start=False, stop=False)
            nc.tensor.matmul(out=s_ps[:], lhsT=sWq_sb[:], rhs=qeT_in[:],
                             start=False, stop=False)
            nc.tensor.matmul(out=s_ps[:], lhsT=sb_sb[:], rhs=ones_c[:, 0:BC],
                             start=False, stop=True)
            sT = p3_p.tile([F, BC], f32, tag="sT")
            nc.scalar.activation(sT[:], s_ps[:], AF.Tanh)

            o_ps = ps_s.tile([2, BC], f32, tag="o_ps", space="PSUM")
            nc.tensor.matmul(out=o_ps[:], lhsT=thaW_sb[:], rhs=sT[:],
                             start=True, stop=False)
            nc.tensor.matmul(out=o_ps[:], lhsT=aWq2_sb[:], rhs=qeT_in[:],
                             start=False, stop=False)
            nc.tensor.matmul(out=o_ps[:], lhsT=hb_sb[:], rhs=ones_c[:, 0:BC],
                             start=False, stop=True)
            o_sb = p3_p.tile([2, BC], f32, tag="o_sb")
            nc.scalar.copy(o_sb[:], o_ps[:])
            for tl in range(NTL):
                tpo = ps_tp.tile([128, 128], f32, tag="tp", space="PSUM")
                nc.tensor.transpose(out=tpo[:, 0:2],
                                    in_=o_sb[:, tl * 128:(tl + 1) * 128],
                                    identity=ident[0:2, 0:2])
                nc.scalar.copy(thal_sb[:, tl, t], tpo[:, 0:2])

        fin_p = ctx.enter_context(tc.tile_pool(name="fin", bufs=1))
        al_lin = thal_sb[:, :, :, 1]
        th_sb = fin_p.tile([128, NTL, S], f32)
        nc.scalar.copy(th_sb[:], thal_sb[:, :, :, 0])
        th_v = th_sb[:]
        # softplus(x) = ln(exp(x) + 1) — Softplus has no ACT table on TRN2
        al_sb = fin_p.tile([128, NTL, S], f32)
        nc.scalar.activation(al_sb[:], al_lin, AF.Exp)
        nc.scalar.activation(al_sb[:], al_sb[:], AF.Ln, bias=1.0)
        y_sb = fin_p.tile([128, NTL, S], f32)
        nc.vector.scalar_tensor_tensor(out=y_sb[:], in0=th_v, scalar=ABILITY_SCALE,
                                       in1=beta_sb[:], op0=OP.mult, op1=OP.subtract)
        z_sb = fin_p.tile([128, NTL, S], f32)
        nc.vector.tensor_tensor(out=z_sb[:], in0=y_sb[:], in1=al_sb[:], op=OP.mult)
        p_sb = fin_p.tile([128, NTL, S], f32)
        nc.scalar.activation(p_sb[:], z_sb[:], AF.Sigmoid)

        nc.sync.dma_start(p_o[:].rearrange("(tl b) s -> b tl s", tl=NTL), p_sb[:])
        nc.sync.dma_start(th_o[:].rearrange("(tl b) s -> b tl s", tl=NTL), th_v)
        nc.sync.dma_start(be_o[:].rearrange("(tl b) s -> b tl s", tl=NTL), beta_sb[:])
        nc.sync.dma_start(al_o[:].rearrange("(tl b) s -> b tl s", tl=NTL), al_sb[:])
        nc.sync.dma_start(z_o[:].rearrange("(tl b) s -> b tl s", tl=NTL), z_sb[:])

    nc.compile()
    return nc


def _make_in_maps(inputs, S=S, ncores=NCORES):
    qd = np.ascontiguousarray(np.asarray(inputs["q_data"], dtype=np.int32)[:, :S])
    qad = np.ascontiguousarray(np.asarray(inputs["qa_data"], dtype=np.int32)[:, :S])
    f = lambda k: np.ascontiguousarray(np.asarray(inputs[k], dtype=np.float32))
    shared = dict(
        q_emb=f("q_embed_w"),
        qa_emb=f("qa_embed_w"),
        init_vm=f("init_value_memory").T.reshape(1, V * M),
        eaW=np.concatenate([f("erase_W"), f("add_W")], axis=1),
        eab=np.concatenate([f("erase_b"), f("add_b")])[None, :],
        kb=np.concatenate([f("key_memory").T, f("beta_W")], axis=1),
        kbb=np.concatenate([np.zeros(M, np.float32), f("beta_b")])[None, :],
        sWr=f("summary_W")[:V],
        sWq=f("summary_W")[V:],
        sbias=f("summary_b")[None, :],
        thaW=np.concatenate([f("theta_W"), f("alpha_W")[:F]], axis=1),
        aWq2=np.concatenate([np.zeros((K, 1), np.float32), f("alpha_W")[F:]], axis=1),
        hb=np.concatenate([f("theta_b"), f("alpha_b")])[None, :],
    )
    shared = {k: np.ascontiguousarray(v, dtype=np.float32) for k, v in shared.items()}
    in_maps = []
    for c in range(ncores):
        sl = slice(c * BC, (c + 1) * BC)
        m = dict(shared)
        m["q_idx"] = np.ascontiguousarray(qd[sl])
        m["qa_idx"] = np.ascontiguousarray(qad[sl])
        in_maps.append(m)
    return in_maps


_NC_CACHE = {}


def _get_program():
    if "nc" not in _NC_CACHE:
        _NC_CACHE["nc"] = _build_program()
    return _NC_CACHE["nc"]


def kernel(**inputs):
    nc = _get_program()
    in_maps = _make_in_maps(inputs)
    res = run_bass_kernel_spmd(nc, in_maps, core_ids=list(range(NCORES)))
    outs = {}
    for k in ("p_o", "th_o", "be_o", "al_o", "z_o"):
        outs[k] = np.concatenate([r[k] for r in res.results], axis=0)
    return (outs["p_o"], outs["th_o"], outs["be_o"], outs["al_o"], outs["z_o"])


# revision 2
# speedup vs baseline: 1.2488x; 1.2488x over previous
"""DKVMN / DeepIRT bass kernel builder for TRN2.

v3: host pre-gathers embeddings and ships them transposed ([S, feat, batch]),
killing the on-device indirect gathers (Pool.SEQ descriptor generation was a
co-bottleneck) and all phase-1 PE transposes.

Per-core program (data parallel over batch, Bc=256 = 2 tiles of 128):
  per t (interleaved, Tile pipelines across engines):
    phase1: DMA qa_eT/q_eT tiles; PE matmuls for gates
      e=sigmoid(qa@We+eb) [via 0.5+0.5*tanh(x/2)], 1/e = 1+exp(-x),
      a=tanh(qa@Wa+ab), w=softmax(q@K^T+kb) [direct exp, logits are O(0.3)],
      beta=tanh(q@bW+bb).  All ACT funcs sit in one table set.
    scan (DVE):
      scr = Mv*e_bc ; scr -= a_bc ; scr *= w_bc     (= T)
      R = reduce_add_m(scr) ; Mv -= scr
      read = (R + a) * (1/e)          (softmax rows sum to 1)
    heads (PE/ACT, fused in-loop):
      transpose read -> readT tiles; sT = tanh(sWr^T@readT + sWq^T@q_eT + sb)
      [theta; alpha_lin] = thaW^T@sT + aWq2^T@q_eT + hb -> transpose back.
  finals: alpha=softplus=ln(exp(x)+1), z=alpha*(3*theta-beta), p=sigmoid(z).
"""
import sys

for _p in ("/opt/trn_rl_repo", "/root/.axon_site/_ro/trn_rl_repo"):
    if _p not in sys.path:
        sys.path.append(_p)

import numpy as np
from contextlib import ExitStack

import concourse.bacc as bacc
import concourse.bass as bass
import concourse.tile as tile
from concourse import mybir
from concourse.masks import make_identity

f32 = mybir.dt.float32
i32 = mybir.dt.int32
AF = mybir.ActivationFunctionType
OP = mybir.AluOpType

B, S_FULL = 2048, 200
NQ = 10000
M, K, V, F = 50, 50, 200, 50
ABILITY_SCALE = 3.0
NCORES = 8
BC = B // NCORES
NTL = BC // 128


def build_program(S=S_FULL, num_devices=NCORES):
    nc = bacc.Bacc("TRN2", target_bir_lowering=False, debug=False,
                   num_devices=num_devices)

    qa_eT_d = nc.dram_tensor("qa_eT", [S, V, BC], f32, kind="ExternalInput")
    q_eT_d = nc.dram_tensor("q_eT", [S, K, BC], f32, kind="ExternalInput")
    init_vm = nc.dram_tensor("init_vm", [1, V * M], f32, kind="ExternalInput")
    eaW = nc.dram_tensor("eaW", [V, 2 * V], f32, kind="ExternalInput")
    eab = nc.dram_tensor("eab", [1, 2 * V], f32, kind="ExternalInput")
    kb = nc.dram_tensor("kb", [K, M + 1], f32, kind="ExternalInput")
    kbb = nc.dram_tensor("kbb", [1, M + 1], f32, kind="ExternalInput")
    sWr = nc.dram_tensor("sWr", [V, F], f32, kind="ExternalInput")
    sWq = nc.dram_tensor("sWq", [K, F], f32, kind="ExternalInput")
    sbias = nc.dram_tensor("sbias", [1, F], f32, kind="ExternalInput")
    thaW = nc.dram_tensor("thaW", [F, 2], f32, kind="ExternalInput")
    aWq2 = nc.dram_tensor("aWq2", [K, 2], f32, kind="ExternalInput")
    hb = nc.dram_tensor("hb", [1, 2], f32, kind="ExternalInput")

    p_o = nc.dram_tensor("p_o", [BC, S], f32, kind="ExternalOutput")
    th_o = nc.dram_tensor("th_o", [BC, S], f32, kind="ExternalOutput")
    be_o = nc.dram_tensor("be_o", [BC, S], f32, kind="ExternalOutput")
    al_o = nc.dram_tensor("al_o", [BC, S], f32, kind="ExternalOutput")
    z_o = nc.dram_tensor("z_o", [BC, S], f32, kind="ExternalOutput")

    with ExitStack() as ctx:
        tc = ctx.enter_context(tile.TileContext(nc))

        const_p = ctx.enter_context(tc.tile_pool(name="const", bufs=1))
        state_p = ctx.enter_context(tc.tile_pool(name="state", bufs=1))

        ident = const_p.tile([128, 128], f32)
        make_identity(nc, ident[:])
        ones_c = const_p.tile([1, 512], f32)
        nc.gpsimd.memset(ones_c[:], 1.0)

        eaW_a = const_p.tile([128, 2 * V], f32)
        nc.sync.dma_start(eaW_a[:], eaW[0:128, :])
        eaW_b = const_p.tile([128, 2 * V], f32)
        nc.sync.dma_start(eaW_b[: V - 128, :], eaW[128:V, :])
        eab_sb = const_p.tile([1, 2 * V], f32)
        nc.sync.dma_start(eab_sb[:], eab[:])
        kb_sb = const_p.tile([K, M + 1], f32)
        nc.sync.dma_start(kb_sb[:], kb[:])
        kbb_sb = const_p.tile([1, M + 1], f32)
        nc.sync.dma_start(kbb_sb[:], kbb[:])
        sWr_a = const_p.tile([128, F], f32)
        nc.sync.dma_start(sWr_a[:], sWr[0:128, :])
        sWr_b = const_p.tile([128, F], f32)
        nc.sync.dma_start(sWr_b[: V - 128, :], sWr[128:V, :])
        sWq_sb = const_p.tile([K, F], f32)
        nc.sync.dma_start(sWq_sb[:], sWq[:])
        sb_sb = const_p.tile([1, F], f32)
        nc.sync.dma_start(sb_sb[:], sbias[:])
        thaW_sb = const_p.tile([F, 2], f32)
        nc.sync.dma_start(thaW_sb[:], thaW[:])
        aWq2_sb = const_p.tile([K, 2], f32)
        nc.sync.dma_start(aWq2_sb[:], aWq2[:])
        hb_sb = const_p.tile([1, 2], f32)
        nc.sync.dma_start(hb_sb[:], hb[:])

        mv = state_p.tile([128, NTL, V, M], f32)
        nc.sync.dma_start(
            mv[:],
            init_vm[:].rearrange("o (v m) -> o v m", m=M).unsqueeze(1)
            .broadcast_to([128, NTL, V, M]),
        )
        beta_sb = state_p.tile([128, NTL, S], f32)
        thal_sb = state_p.tile([128, NTL, S, 2], f32)
        scr = state_p.tile([128, V, M], f32)

        ps_tp = ctx.enter_context(tc.tile_pool(name="ps_tp", bufs=2, space="PSUM"))
        ps_ea = ctx.enter_context(tc.tile_pool(name="ps_ea", bufs=2, space="PSUM"))
        ps_lg = ctx.enter_context(tc.tile_pool(name="ps_lg", bufs=2, space="PSUM"))
        ps_s = ctx.enter_context(tc.tile_pool(name="ps_s", bufs=1, space="PSUM"))
        ph1_p = ctx.enter_context(tc.tile_pool(name="ph1", bufs=6))
        gates_p = ctx.enter_context(tc.tile_pool(name="gates", bufs=8))
        rd_p = ctx.enter_context(tc.tile_pool(name="rd", bufs=4))

        for t in range(S):
            gates = []
            for tl in range(NTL):
                sl = slice(tl * 128, (tl + 1) * 128)
                qaeT_a = ph1_p.tile([128, 128], f32, tag="qaeT_a")
                nc.sync.dma_start(qaeT_a[:], qa_eT_d[t, 0:128, sl])
                qaeT_b = ph1_p.tile([128, 128], f32, tag="qaeT_b")
                nc.sync.dma_start(qaeT_b[: V - 128, :], qa_eT_d[t, 128:V, sl])
                qeT_sb = ph1_p.tile([K, 128], f32, tag="qeT_sb")
                nc.sync.dma_start(qeT_sb[:], q_eT_d[t, :, sl])

                ea_ps = ps_ea.tile([128, 2 * V], f32, tag="ea", space="PSUM")
                nc.tensor.matmul(out=ea_ps[:], lhsT=qaeT_a[:], rhs=eaW_a[:],
                                 start=True, stop=False)
                nc.tensor.matmul(out=ea_ps[:], lhsT=qaeT_b[: V - 128, :],
                                 rhs=eaW_b[: V - 128, :], start=False, stop=False)
                nc.tensor.matmul(out=ea_ps[:], lhsT=ones_c[:, 0:128],
                                 rhs=eab_sb[:], start=False, stop=True)
                # ACT funcs all in one table set: sigmoid via tanh identity.
                e_t = gates_p.tile([128, V], f32, tag="e_t")
                nc.scalar.activation(e_t[:], ea_ps[:, 0:V], AF.Tanh, scale=0.5)
                nc.scalar.activation(e_t[:], e_t[:], AF.Copy, scale=0.5, bias=0.5)
                re_t = gates_p.tile([128, V], f32, tag="re_t")
                nc.scalar.activation(re_t[:], ea_ps[:, 0:V], AF.Exp, scale=-1.0)
                nc.scalar.activation(re_t[:], re_t[:], AF.Copy, bias=1.0)
                a_t = gates_p.tile([128, V], f32, tag="a_t")
                nc.scalar.activation(a_t[:], ea_ps[:, V:2 * V], AF.Tanh)

                lg_ps = ps_lg.tile([128, M + 1], f32, tag="lg", space="PSUM")
                nc.tensor.matmul(out=lg_ps[:], lhsT=qeT_sb[:], rhs=kb_sb[:],
                                 start=True, stop=False)
                nc.tensor.matmul(out=lg_ps[:], lhsT=ones_c[:, 0:128],
                                 rhs=kbb_sb[:], start=False, stop=True)
                # logits are O(0.3): direct exp softmax, fused sum via accum.
                w_t = gates_p.tile([128, M], f32, tag="w_t")
                sm = rd_p.tile([128, 1], f32, tag="sm")
                nc.scalar.activation(w_t[:], lg_ps[:, 0:M], AF.Exp, accum_out=sm[:])
                rs = rd_p.tile([128, 1], f32, tag="rs")
                nc.vector.reciprocal(rs[:], sm[:])
                nc.vector.tensor_scalar_mul(w_t[:], w_t[:], rs[:])
                nc.scalar.activation(beta_sb[:, tl, t:t + 1], lg_ps[:, M:M + 1],
                                     AF.Tanh)
                gates.append((e_t, a_t, re_t, w_t, qeT_sb))

            for tl in range(NTL):
                e_t, a_t, re_t, w_t, qeT_sb = gates[tl]
                mvv = mv[:, tl]
                e_bc = e_t[:].unsqueeze(2).broadcast_to([128, V, M])
                a_bc = a_t[:].unsqueeze(2).broadcast_to([128, V, M])
                w_bc = w_t[:].unsqueeze(1).broadcast_to([128, V, M])
                nc.vector.tensor_tensor(out=scr[:], in0=mvv, in1=e_bc, op=OP.mult)
                nc.vector.tensor_tensor(out=scr[:], in0=scr[:], in1=a_bc,
                                        op=OP.subtract)
                nc.vector.tensor_tensor(out=scr[:], in0=scr[:], in1=w_bc, op=OP.mult)
                R_t = rd_p.tile([128, V], f32, tag="R_t")
                nc.vector.tensor_reduce(out=R_t[:], in_=scr[:],
                                        axis=mybir.AxisListType.X, op=OP.add)
                nc.vector.tensor_tensor(out=mvv, in0=mvv, in1=scr[:], op=OP.subtract)
                rd_t = rd_p.tile([128, V], f32, tag="rd_t")
                nc.vector.tensor_tensor(out=rd_t[:], in0=R_t[:], in1=a_t[:], op=OP.add)
                nc.vector.tensor_tensor(out=rd_t[:], in0=rd_t[:], in1=re_t[:],
                                        op=OP.mult)

                # heads for (t, tl), fused: SBUF readT/q_eT tiles feed PE
                tp4 = ps_tp.tile([128, 128], f32, tag="tp", space="PSUM")
                nc.tensor.transpose(out=tp4[:], in_=rd_t[:, 0:128], identity=ident[:])
                rT_a = rd_p.tile([128, 128], f32, tag="rT_a")
                nc.scalar.copy(rT_a[:], tp4[:])
                tp5 = ps_tp.tile([128, 128], f32, tag="tp", space="PSUM")
                nc.tensor.transpose(out=tp5[: V - 128, :], in_=rd_t[:, 128:V],
                                    identity=ident[:])
                rT_b = rd_p.tile([128, 128], f32, tag="rT_b")
                nc.scalar.copy(rT_b[: V - 128, :], tp5[: V - 128, :])

                s_ps = ps_s.tile([F, 128], f32, tag="s_ps", space="PSUM")
                nc.tensor.matmul(out=s_ps[:], lhsT=sWr_a[:], rhs=rT_a[:],
                                 start=True, stop=False)
                nc.tensor.matmul(out=s_ps[:], lhsT=sWr_b[: V - 128, :],
                                 rhs=rT_b[: V - 128, :], start=False, stop=False)
                nc.tensor.matmul(out=s_ps[:], lhsT=sWq_sb[:], rhs=qeT_sb[:],
                                 start=False, stop=False)
                nc.tensor.matmul(out=s_ps[:], lhsT=sb_sb[:], rhs=ones_c[:, 0:128],
                                 start=False, stop=True)
                sT = rd_p.tile([F, 128], f32, tag="sT")
                nc.scalar.activation(sT[:], s_ps[:], AF.Tanh)

                o_ps = ps_s.tile([2, 128], f32, tag="o_ps", space="PSUM")
                nc.tensor.matmul(out=o_ps[:], lhsT=thaW_sb[:], rhs=sT[:],
                                 start=True, stop=False)
                nc.tensor.matmul(out=o_ps[:], lhsT=aWq2_sb[:], rhs=qeT_sb[:],
                                 start=False, stop=False)
                nc.tensor.matmul(out=o_ps[:], lhsT=hb_sb[:], rhs=ones_c[:, 0:128],
                                 start=False, stop=True)
                o_sb = rd_p.tile([2, 128], f32, tag="o_sb")
                nc.scalar.copy(o_sb[:], o_ps[:])
                tpo = ps_tp.tile([128, 128], f32, tag="tp", space="PSUM")
                nc.tensor.transpose(out=tpo[:, 0:2], in_=o_sb[:],
                                    identity=ident[0:2, 0:2])
                nc.scalar.copy(thal_sb[:, tl, t], tpo[:, 0:2])

        fin_p = ctx.enter_context(tc.tile_pool(name="fin", bufs=1))
        al_lin = thal_sb[:, :, :, 1]
        th_sb = fin_p.tile([128, NTL, S], f32)
        nc.scalar.copy(th_sb[:], thal_sb[:, :, :, 0])
        th_v = th_sb[:]
        # softplus(x) = ln(exp(x) + 1) — Softplus has no ACT table on TRN2
        al_sb = fin_p.tile([128, NTL, S], f32)
        nc.scalar.activation(al_sb[:], al_lin, AF.Exp)
        nc.scalar.activation(al_sb[:], al_sb[:], AF.Ln, bias=1.0)
        y_sb = fin_p.tile([128, NTL, S], f32)
        nc.vector.scalar_tensor_tensor(out=y_sb[:], in0=th_v, scalar=ABILITY_SCALE,
                                       in1=beta_sb[:], op0=OP.mult, op1=OP.subtract)
        z_sb = fin_p.tile([128, NTL, S], f32)
        nc.vector.tensor_tensor(out=z_sb[:], in0=y_sb[:], in1=al_sb[:], op=OP.mult)
        p_sb = fin_p.tile([128, NTL, S], f32)
        nc.scalar.activation(p_sb[:], z_sb[:], AF.Sigmoid)

        nc.sync.dma_start(p_o[:].rearrange("(tl b) s -> b tl s", tl=NTL), p_sb[:])
        nc.sync.dma_start(th_o[:].rearrange("(tl b) s -> b tl s", tl=NTL), th_v)
        nc.sync.dma_start(be_o[:].rearrange("(tl b) s -> b tl s", tl=NTL), beta_sb[:])
        nc.sync.dma_start(al_o[:].rearrange("(tl b) s -> b tl s", tl=NTL), al_sb[:])
        nc.sync.dma_start(z_o[:].rearrange("(tl b) s -> b tl s", tl=NTL), z_sb[:])

    nc.compile()
    return nc


def make_in_maps(inputs, S=S_FULL, ncores=NCORES):
    qd = np.asarray(inputs["q_data"]).astype(np.int64)[:, :S]
    qad = np.asarray(inputs["qa_data"]).astype(np.int64)[:, :S]
    f = lambda k: np.ascontiguousarray(np.asarray(inputs[k], dtype=np.float32))
    q_emb = f("q_embed_w")
    qa_emb = f("qa_embed_w")
    # host pre-gather + transpose to [S, feat, B]
    q_eT = np.ascontiguousarray(q_emb[qd].transpose(1, 2, 0))     # [S, K, B]
    qa_eT = np.ascontiguousarray(qa_emb[qad].transpose(1, 2, 0))  # [S, V, B]
    shared = dict(
        init_vm=f("init_value_memory").T.reshape(1, V * M),
        eaW=np.concatenate([f("erase_W"), f("add_W")], axis=1),
        eab=np.concatenate([f("erase_b"), f("add_b")])[None, :],
        kb=np.concatenate([f("key_memory").T, f("beta_W")], axis=1),
        kbb=np.concatenate([np.zeros(M, np.float32), f("beta_b")])[None, :],
        sWr=f("summary_W")[:V],
        sWq=f("summary_W")[V:],
        sbias=f("summary_b")[None, :],
        thaW=np.concatenate([f("theta_W"), f("alpha_W")[:F]], axis=1),
        aWq2=np.concatenate([np.zeros((K, 1), np.float32), f("alpha_W")[F:]], axis=1),
        hb=np.concatenate([f("theta_b"), f("alpha_b")])[None, :],
    )
    shared = {k: np.ascontiguousarray(v, dtype=np.float32) for k, v in shared.items()}
    in_maps = []
    for c in range(ncores):
        sl = slice(c * BC, (c + 1) * BC)
        m = dict(shared)
        m["q_eT"] = np.ascontiguousarray(q_eT[:, :, sl])
        m["qa_eT"] = np.ascontiguousarray(qa_eT[:, :, sl])
        in_maps.append(m)
    return in_maps


def assemble_outputs(results, S=S_FULL):
    outs = {}
    for k in ("p_o", "th_o", "be_o", "al_o", "z_o"):
        outs[k] = np.concatenate([r[k] for r in results], axis=0)
    return (outs["p_o"], outs["th_o"], outs["be_o"], outs["al_o"], outs["z_o"])


from concourse.bass_utils import run_bass_kernel_spmd

_NC_CACHE = {}


def _get_program():
    if "nc" not in _NC_CACHE:
        _NC_CACHE["nc"] = build_program()
    return _NC_CACHE["nc"]


# aliases used by test harnesses
_build_program = build_program
_make_in_maps = make_in_maps


def kernel(**inputs):
    """Full-input entry point: shards batch over 8 NeuronCores, returns the
    reference-matching tuple (p, theta, beta, alpha, z), each [2048, 200] f32."""
    nc = _get_program()
    in_maps = make_in_maps(inputs)
    res = run_bass_kernel_spmd(nc, in_maps, core_ids=list(range(NCORES)))
    return assemble_outputs(res.results)
